# revision 1
# baseline (speedup 1.0000x reference)
r"""Circulant layer kernel for Trainium2 (8 NeuronCores).

Math: reference computes mv1 + mv2 where
  mv1 = batch_circulant(b) @ d,  mv2 = batch_circulant(d) @ b,
with d = des @ K, b = body @ K.  Both are the circular convolution of d and b
(circular convolution is commutative), so  out = 2 * circconv(d, b).

circconv via DFT:  out = 2 * Re(IDFT(DFT(d) * DFT(b))).
DFT/IDFT are realized as dense matmuls with host-generated constant
cos/sin matrices (input-independent constants).

Sharding: each of the 8 cores owns 128 of the 1024 DFT frequencies.
Per core c:
  KC_c   = K @ CC_c            (1024k x 256s)   fused projection+forward DFT
  DT_c   = KC_c^T @ des^T      (256s x 128b)    \  shares stationary weights
  BT_c   = KC_c^T @ body^T     (256s x 128b)    /
  PT_c   = complex-mult(DT_c, BT_c)             (256s x 128b)  on VectorE
  part_c = (PT_c^T @ G_c)                       (128b x 1024)  inverse DFT
Host sums the 8 partials (unshard).
"""

import numpy as np

import concourse.bass as bass
import concourse.mybir as mybir
import concourse.tile as tile
from concourse.bass_utils import run_bass_kernel_spmd
from concourse.tile_rust import add_dep_helper

B = 128        # batch
D_IN = 1024    # input feature dim (contraction k)
N = 1024       # output feature dim (conv length j) == #frequencies
N_CORES = 8
FPC = N // N_CORES  # frequencies per core (complex)
S = 2 * FPC         # freq slots per core: [0:FPC]=real(cos), [FPC:2FPC]=imag(-sin)

F32 = mybir.dt.float32
F32R = mybir.dt.float32r
BF16 = mybir.dt.bfloat16

# Matmul operand precision: "bf16" (fastest; ~5e-3 rel err), "f32r"
# (single-pass TF32-like; ~3e-4), "f32" (two-pass full fp32; ~7e-7).
import os as _os
MM_PREC = _os.environ.get("CIRC_MM_PREC", "f32r")
MM_DT = {"bf16": BF16, "f32r": F32R, "f32": F32}[MM_PREC]


def _np_in(a):
    """Cast to the matmul precision; bf16 data is shipped packed in fp32
    words (DMA is element-rate-bound: 2-byte elements run at half rate)."""
    import ml_dtypes
    a = np.ascontiguousarray(np.asarray(a, dtype=np.float32))
    if MM_PREC != "bf16":
        return a
    bf = np.ascontiguousarray(a.astype(ml_dtypes.bfloat16))
    return bf.view(np.uint8).reshape(a.shape[0], -1).view(np.float32)

# Number of fp32 transport words per logical input element.
PACK = 2 if MM_PREC == "bf16" else 1
# Transport dtype: bf16 ships packed in fp32 words; f32/f32r ship natively
# (the fp32r verifier requires the producing DMA to be f32r-typed).
TR_DT = F32 if MM_PREC == "bf16" else MM_DT

# Stashed by kernel() for test harnesses that want profiling info.
LAST_RESULT = None

_nc_cache = {}


def _build_nc():
    """Build the (single-program) Bass module run on all 8 cores."""
    nc = bass.Bass(target_bir_lowering=True)

    # Packed inputs: tensors consumed together share one DMA (keeps the
    # per-matmul semaphore-wait count within the ISA limit).
    #   ktcc[j, :D_IN] = K^T,  ktcc[j, D_IN:] = CC   (both indexed by j)
    #   dbt[k, :B] = des^T,    dbt[k, B:] = body^T   (both indexed by k)
    # All inputs are host-packed per SBUF partition: row p holds everything
    # partition p receives, contiguously, so each DMA moves 128 long
    # contiguous rows (DMA throughput is descriptor-rate-bound otherwise).
    XW = (D_IN + S) // PACK
    DW = 2 * B // PACK
    GW = N // PACK
    # ktcc in two contiguous halves, one per HWDGE channel (SP / ACT).
    # Channels are FIFO with ~4-5us latency PER TRANSFER, so one big
    # transfer per channel beats several small ones.
    ktcc_q = [nc.declare_dram_parameter(f"ktcc{i}", [128, 4 * XW], TR_DT, False)
              for i in range(2)]
    # aux = [dbt rows | g rows] packed per partition; goes via gpsimd SWDGE.
    aux = nc.declare_dram_parameter("aux", [128, 8 * DW + 2 * GW], TR_DT, False)  # (s, t) inv DFT rows
    out = nc.declare_dram_parameter("out", [B, N], F32, isOutput=True)
    warm_scratch = nc.dram_tensor("warm_scratch", [1, 4], F32)

    JC = N // 128      # 8 chunks over j (contraction of KC stage)
    KB = D_IN // 128   # 8 blocks over k (output partitions of KC stage)
    SB = S // 128      # 2 blocks over freq slots

    with tile.TileContext(nc) as tc:
        with (
            tc.tile_pool(name="main", bufs=1) as pool,
            tc.tile_pool(name="psum", bufs=1, space="PSUM") as pp,
        ):
            # ---- inputs -> SBUF ----
            # At most 7 input DMAs so the output store lands on the 8th,
            # otherwise-unused HW queue: a DMACopy can carry only ONE sync
            # wait, and the store needs its data-dependency wait — it must
            # not also need a queue-slot wait.
            in_dmas = []
            # All input transfers ride ONE serial SP chain: parallel channels
            # all pay the full ~12us proxy latency, while a serial chain
            # pipelines (first chunk lands ~6us in, rest follow every ~3us).
            ktcc_sb = [pool.tile([128, 4, XW], TR_DT, tag=f"ktcc{q}", name=f"ktcc{q}")
                       for q in range(2)]
            for q in range(2):
                in_dmas.append(nc.sync.dma_start(ktcc_sb[q][:], ktcc_q[q][:, :]))
            aux_raw = pool.tile([128, 8 * DW + 2 * GW], TR_DT, tag="auxr", name="auxr")
            in_dmas.append(nc.sync.dma_start(aux_raw[:], aux[:, :]))
            ktcc_v = [t.bitcast(MM_DT) for t in ktcc_sb]
            kt_sb = [ktcc_v[j // 4][:, j % 4, :D_IN] for j in range(JC)]
            cc_sb = [ktcc_v[j // 4][:, j % 4, D_IN:] for j in range(JC)]
            dbt_sb = pool.tile([128, KB, DW], TR_DT, tag="dbt", name="dbt")
            nc.vector.tensor_copy(dbt_sb[:], aux_raw[:, :8 * DW].rearrange("p (kb w) -> p kb w", kb=KB))
            dbt_v = dbt_sb.bitcast(MM_DT)
            g_stage = pool.tile([128, SB, GW], TR_DT, tag="gst", name="gst")
            nc.vector.tensor_copy(g_stage[:], aux_raw[:, 8 * DW:].rearrange("p (sb w) -> p sb w", sb=SB))
            g_sb = [g_stage.bitcast(MM_DT)[:, s, :] for s in range(SB)]

            # ---- PE warmup: keep the HAM clock un-throttled while DMAs
            # stream in, so the real matmuls all run at 2.4 GHz. Dead-code
            # proofed by a tiny gpsimd DMA of the result to scratch DRAM.
            wz = pool.tile([128, 640], BF16, tag="wz", name="wz")
            nc.gpsimd.memset(wz[:], 0.0)
            wps = pp.tile([128, 512], F32, tag="wps", name="wps")
            for w in range(20):
                nc.tensor.matmul(wps[:], wz[:, :128], wz[:, 128:640],
                                 start=True, stop=True)
            wsb = pool.tile([128, 4], F32, tag="wsb", name="wsb")
            nc.vector.tensor_copy(wsb[:], wps[:, :4])
            warm_dma = nc.gpsimd.dma_start(warm_scratch[:, :], wsb[:1, :])

            # ---- stage 1 + stage 2 interleaved ----
            # KC[k, s] = sum_j KT[j, k] * CC[j, s]; as soon as kc chunk kb is
            # cast to bf16, both stage-2 accumulations consume it, hiding the
            # stage-2 matmuls inside stage-1's DMA-paced gaps.
            kc_sb = [pool.tile([128, S], MM_DT, tag=f"kc{kb}", name=f"kc{kb}") for kb in range(KB)]
            db_all = pool.tile([128, SB, 2 * B], F32, tag="dball", name="dball")
            db_ps = [pp.tile([128, 2 * B], F32, tag=f"dbp{sb}", name=f"dbp{sb}")
                     for sb in range(SB)]
            for kb in range(KB):
                ps = pp.tile([128, S], F32, tag="kcp", name=f"kcp{kb}", bufs=2)
                for j in range(JC):
                    nc.tensor.matmul(
                        ps[:],
                        kt_sb[j][:, kb * 128:(kb + 1) * 128],
                        cc_sb[j][:],
                        start=(j == 0),
                        stop=(j == JC - 1),
                    )
                nc.vector.tensor_copy(kc_sb[kb][:], ps[:])
                if MM_PREC == "bf16":
                    # interleave stage-2 into stage-1's DMA-paced gaps; for
                    # f32/f32r the serialized 4-byte weight loads make this
                    # interleave a net loss, so run stage 2 afterwards.
                    for sb in range(SB):
                        nc.tensor.matmul(db_ps[sb][:],
                                         kc_sb[kb][:, sb * 128:(sb + 1) * 128],
                                         dbt_v[:, kb, :],
                                         start=(kb == 0), stop=(kb == KB - 1))
            if MM_PREC != "bf16":
                for sb in range(SB):
                    for kb in range(KB):
                        nc.tensor.matmul(db_ps[sb][:],
                                         kc_sb[kb][:, sb * 128:(sb + 1) * 128],
                                         dbt_v[:, kb, :],
                                         start=(kb == 0), stop=(kb == KB - 1))
            for sb in range(SB):
                nc.vector.tensor_copy(db_all[:, sb, :], db_ps[sb][:])

            # ---- stage 3: complex pointwise multiply (on freq partitions) ----
            # t01 = [Dr*Br, Dr*Bi], t23 = [Di*Bi, Di*Br]
            # Pr = t01[0] - t23[0],  Pi = t01[1] + t23[1]
            t01 = pool.tile([128, 2, B], F32, tag="t01", name="t01")
            t23 = pool.tile([128, 2, B], F32, tag="t23", name="t23")
            pt = pool.tile([128, 2, B], MM_DT, tag="pt", name="pt")
            dr_b = db_all[:, 0, :B][:, None, :].to_broadcast((128, 2, B))
            di_b = db_all[:, 1, :B][:, None, :].to_broadcast((128, 2, B))
            nc.vector.tensor_mul(t01[:], dr_b, db_all[:, :, B:])
            nc.vector.tensor_mul(t23[:], di_b, db_all[:, ::-1, B:])
            nc.vector.tensor_sub(pt[:, 0, :], t01[:, 0, :], t23[:, 0, :])
            nc.vector.tensor_add(pt[:, 1, :], t01[:, 1, :], t23[:, 1, :])
            pt_sb = [pt[:, sb, :] for sb in range(SB)]

            # ---- stage 4: part = PT^T @ G ----
            out_sb = pool.tile([128, N], F32, tag="outsb", name="outsb")
            last_mm = last_cp = None
            for h in range(2):
                o_ps = pp.tile([128, 512], F32, tag="op", name=f"op{h}", bufs=2)
                for sb in range(SB):
                    last_mm = nc.tensor.matmul(
                        o_ps[:],
                        pt_sb[sb],
                        g_sb[sb][:, h * 512:(h + 1) * 512],
                        start=(sb == 0),
                        stop=(sb == SB - 1),
                    )
                last_cp = nc.vector.tensor_copy(out_sb[:, h * 512:(h + 1) * 512], o_ps[:])
            store_a = nc.sync.dma_start(out[:, :512], out_sb[:, :512])
            store_b = nc.scalar.dma_start(out[:, 512:], out_sb[:, 512:])

            # TileContext's exit emits one tail Drain waiting on every
            # outstanding semaphore; walrus caps instructions at ONE sync
            # wait.  Pre-absorb every tick into SP's clock with a chain of
            # single-wait drains so the tail drain needs none.
            prev = None
            for dep in [*in_dmas, warm_dma, store_a, store_b, last_mm, last_cp]:
                dr = nc.sync.drain(fusable=False)
                add_dep_helper(dr.ins, dep.ins, sync=True,
                               reason="tail: absorb tick into SP clock")
                if prev is not None:
                    add_dep_helper(dr.ins, prev.ins, sync=False,
                                   reason="tail: keep drain chain ordered")
                prev = dr

    return nc


def _dft_constants():
    """Per-core forward (CC) and inverse (G) DFT matrices, float32."""
    j = np.arange(N, dtype=np.float64)
    ccs, gs = [], []
    for c in range(N_CORES):
        f = np.arange(c * FPC, (c + 1) * FPC, dtype=np.float64)
        ang = 2.0 * np.pi * np.outer(j, f) / N          # (j, f)
        cc = np.concatenate([np.cos(ang), -np.sin(ang)], axis=1)   # (N, S)
        # inverse: out[k] = (2/N) * sum_f [Pr cos(2pi f k/N) - Pi sin(2pi f k/N)]
        angT = ang.T                                     # (f, k)
        gr = (2.0 / N) * np.cos(angT)
        gi = -(2.0 / N) * np.sin(angT)
        gmat = np.concatenate([gr, gi], axis=0)          # (S, N)
        ccs.append(np.ascontiguousarray(cc, dtype=np.float32))
        gs.append(np.ascontiguousarray(gmat, dtype=np.float32))
    return ccs, gs


def _partition_pack(a):
    """(R, W) with R = n*128 -> (128, n*W): row p = concat of chunk rows p."""
    r, w = a.shape
    n = r // 128
    return np.ascontiguousarray(
        a.reshape(n, 128, w).transpose(1, 0, 2).reshape(128, n * w))


def kernel(des, body, kernel):
    global LAST_RESULT
    K = np.asarray(kernel, dtype=np.float32)
    kt_np = K.T  # (j, k)
    dbt_np = _partition_pack(_np_in(np.concatenate(
        [np.asarray(des, dtype=np.float32).T, np.asarray(body, dtype=np.float32).T],
        axis=1,
    )))  # (k, 2B) packed
    ccs, gs = _dft_constants()
    ktccs = [
        _partition_pack(_np_in(np.concatenate([kt_np, ccs[c]], axis=1)))
        for c in range(N_CORES)
    ]
    half = ktccs[0].shape[1] // 2
    auxs = [
        np.ascontiguousarray(
            np.concatenate([dbt_np, _partition_pack(_np_in(gs[c]))], axis=1))
        for c in range(N_CORES)
    ]

    if "nc" not in _nc_cache:
        _nc_cache["nc"] = _build_nc()
    nc = _nc_cache["nc"]

    in_maps = [
        {**{f"ktcc{i}": np.ascontiguousarray(ktccs[c][:, i * half:(i + 1) * half])
            for i in range(2)},
         "aux": auxs[c]}
        for c in range(N_CORES)
    ]
    res = run_bass_kernel_spmd(nc, in_maps, list(range(N_CORES)))
    LAST_RESULT = res
    out = np.zeros((B, N), dtype=np.float32)
    for r in res.results:
        out += r["out"]
    return out



# revision 7
# speedup vs baseline: 1.0210x; 1.0210x over previous
r"""Circulant layer kernel for Trainium2 (8 NeuronCores).

Math: reference computes mv1 + mv2 where
  mv1 = batch_circulant(b) @ d,  mv2 = batch_circulant(d) @ b,
with d = des @ K, b = body @ K.  Both are the circular convolution of d and b
(circular convolution is commutative), so  out = 2 * circconv(d, b).

circconv via DFT:  out = 2 * Re(IDFT(DFT(d) * DFT(b))).
DFT/IDFT are realized as dense matmuls with host-generated constant
cos/sin matrices (input-independent constants).

Real-input symmetry: d, b are real so the spectrum is conjugate-symmetric;
only frequencies 0..512 are computed, with inverse-DFT weights 4/N for
f=1..511 and 2/N for f=0 and f=512 (Nyquist).  513 = 8*64 + 1 frequencies
are spread as 65 slots per core (uniform program): slot 64 is a zero pad on
cores 0..6 and the Nyquist bin on core 7 (whose -sin column is identically
zero, so the complex pointwise multiply degenerates correctly).

Sharding: core c owns frequencies 64c..64c+63 (+pad).  Per core:
  KC_c  = K @ CC_c          (1024k x 130s)   fused projection+forward DFT,
                            j-outer so matmuls ride the K DMA stream
  DT_c  = KC_c^T @ [des^T body^T]  (65s x 256)  x2 (cos/sin blocks)
  PT_c  = complex-mult(DT)  (65s x 128b) x2   on VectorE
  part_c = PT_c^T @ G_c     (128b x 1024)     inverse DFT, contraction s=65
Host sums the 8 partials (unshard).
"""

import numpy as np

import concourse.bass as bass
import concourse.mybir as mybir
import concourse.tile as tile
from concourse.bass_utils import run_bass_kernel_spmd
from concourse.tile_rust import add_dep_helper

B = 128        # batch
D_IN = 1024    # input feature dim (contraction k)
N = 1024       # output feature dim (conv length j) == #frequencies
N_CORES = 8
NFREQ = 64          # real frequencies per core
NF = NFREQ + 1      # slots per block (incl pad/Nyquist)
S = 2 * NF          # total freq slots per core: [0:NF]=cos, [NF:2NF]=-sin

F32 = mybir.dt.float32
F32R = mybir.dt.float32r
BF16 = mybir.dt.bfloat16

# Matmul operand precision: "bf16" (fastest; ~5e-3 rel err), "f32r"
# (single-pass TF32-like; ~3e-4), "f32" (two-pass full fp32; ~7e-7).
import os as _os
MM_PREC = _os.environ.get("CIRC_MM_PREC", "bf16")
MM_DT = {"bf16": BF16, "f32r": F32R, "f32": F32}[MM_PREC]


def _np_in(a):
    """Cast to the matmul precision; bf16 data is shipped packed in fp32
    words (DMA is element-rate-bound: 2-byte elements run at half rate)."""
    import ml_dtypes
    a = np.ascontiguousarray(np.asarray(a, dtype=np.float32))
    if MM_PREC != "bf16":
        return a
    bf = np.ascontiguousarray(a.astype(ml_dtypes.bfloat16))
    return bf.view(np.uint8).reshape(a.shape[0], -1).view(np.float32)

# Number of fp32 transport words per logical input element.
PACK = 2 if MM_PREC == "bf16" else 1
# Transport dtype: bf16 ships packed in fp32 words; f32/f32r ship natively
# (the fp32r verifier requires the producing DMA to be f32r-typed).
TR_DT = F32 if MM_PREC == "bf16" else MM_DT

# Stashed by kernel() for test harnesses that want profiling info.
LAST_RESULT = None

_nc_cache = {}

JC = N // 128      # 8 chunks over j (contraction of KC stage)
KB = D_IN // 128   # 8 blocks over k (output partitions of KC stage)
KT_SPLIT = 4       # K^T arrives in this many DMA transfers (j-chunk pairs)


def _build_nc():
    """Build the (single-program) Bass module run on all 8 cores."""
    nc = bass.Bass(target_bir_lowering=True)

    # Input packing (all host-packed per SBUF partition; every DMA moves
    # long contiguous rows — DMA throughput is descriptor-rate-bound
    # otherwise).  All input transfers ride ONE serial SP chain: parallel
    # channels all pay the full proxy latency, while a serial chain
    # pipelines.  Transfer order = consumption order:
    #   ccdbt[p, ic, :]  = CC chunk ic rows | des^T/body^T chunk ic rows
    #   kt{q}[p, h, :]   = K^T rows for j-chunk 2q+h  (stage-1 stream)
    #   g[s, blk, :]     = inverse-DFT rows (cos/sin blocks), s=0..NF-1
    CW = S // PACK            # CC words per chunk row
    DW = 2 * B // PACK        # dbt words per chunk row
    GW = 2 * N // PACK        # g words per slot row (cos+sin blocks)
    ccdbt = nc.declare_dram_parameter("ccdbt", [128, JC * (CW + DW)], TR_DT, False)
    kt_q = [nc.declare_dram_parameter(f"kt{q}", [128, (JC // KT_SPLIT) * D_IN // PACK],
                                      TR_DT, False)
            for q in range(KT_SPLIT)]
    g_par = nc.declare_dram_parameter("g", [NF, GW], TR_DT, False)
    out = nc.declare_dram_parameter("out", [B, N // PACK], F32, isOutput=True)
    warm_scratch = nc.dram_tensor("warm_scratch", [1, 4], F32)

    JPT = JC // KT_SPLIT   # j-chunks per kt transfer

    with tile.TileContext(nc) as tc:
        with (
            tc.tile_pool(name="main", bufs=1) as pool,
            tc.tile_pool(name="psum", bufs=1, space="PSUM") as pp,
        ):
            # ---- inputs -> SBUF (one serial SP chain) ----
            in_dmas = []
            ccdbt_sb = pool.tile([128, JC, CW + DW], TR_DT, tag="ccdbt", name="ccdbt")
            in_dmas.append(nc.sync.dma_start(ccdbt_sb[:], ccdbt[:, :]))
            kt_sb = [pool.tile([128, JPT, D_IN // PACK], TR_DT, tag=f"kt{q}", name=f"kt{q}")
                     for q in range(KT_SPLIT)]
            for q in range(KT_SPLIT):
                in_dmas.append(nc.sync.dma_start(kt_sb[q][:], kt_q[q][:, :]))
            g_sb = pool.tile([NF, GW], TR_DT, tag="g", name="g")
            in_dmas.append(nc.sync.dma_start(g_sb[:], g_par[:, :]))

            ccdbt_v = ccdbt_sb.bitcast(MM_DT)          # [128, JC, S + 2B]
            cc_sb = [ccdbt_v[:, j, :S] for j in range(JC)]
            dbt_sb = [ccdbt_v[:, kb, S:] for kb in range(KB)]
            kt_v = [t.bitcast(MM_DT) for t in kt_sb]   # [128, JPT, D_IN]
            g_v = g_sb.bitcast(MM_DT)                  # [NF, 2N]

            # ---- PE warmup: keep the HAM clock un-throttled while DMAs
            # stream in, so the real matmuls all run at 2.4 GHz. Dead-code
            # proofed by a tiny gpsimd DMA of the result to scratch DRAM.
            wz = pool.tile([128, 640], BF16, tag="wz", name="wz")
            nc.gpsimd.memset(wz[:], 0.0)
            wps = pp.tile([128, 512], F32, tag="op", name="wps", bufs=2)
            for w in range(16):
                nc.tensor.matmul(wps[:], wz[:, :128], wz[:, 128:640],
                                 start=True, stop=True)
            wsb = pool.tile([128, 4], F32, tag="wsb", name="wsb")
            nc.vector.tensor_copy(wsb[:], wps[:, :4])
            warm_dma = nc.gpsimd.dma_start(warm_scratch[:, :], wsb[:1, :])

            # ---- stage 1, j-outer: KC[k, s] = sum_j KT[j, k] * CC[j, s].
            # All 8 k-blocks accumulate in PSUM simultaneously so each
            # j-chunk's matmuls issue as soon as that chunk's K^T lands.
            # PSUM is 8 banks of 2KB/partition and accumulation groups may
            # not share a bank, so stage 1 runs in two passes of 4 k-blocks
            # (4 banks, reused), each pass j-outer over the K^T stream.
            # Stage 2 consumes each k-block right after its group closes.
            kc_ps = [pp.tile([128, S], F32, tag=f"kcp{q}", name=f"kcp{q}")
                     for q in range(4)]
            kc_sb = [pool.tile([128, S], MM_DT, tag=f"kc{kb}", name=f"kc{kb}")
                     for kb in range(KB)]
            db_ps = [pp.tile([NF, 2 * B], F32, tag=f"dbp{h}", name=f"dbp{h}")
                     for h in range(2)]
            for half in range(2):
                for j in range(JC):
                    ktj = kt_v[j // JPT][:, j % JPT, :]
                    for q in range(4):
                        kb = half * 4 + q
                        nc.tensor.matmul(
                            kc_ps[q][:],
                            ktj[:, kb * 128:(kb + 1) * 128],
                            cc_sb[j],
                            start=(j == 0),
                            stop=(j == JC - 1),
                        )
                for q in range(4):
                    kb = half * 4 + q
                    nc.vector.tensor_copy(kc_sb[kb][:], kc_ps[q][:])
                    for h in range(2):
                        nc.tensor.matmul(db_ps[h][:],
                                         kc_sb[kb][:, h * NF:(h + 1) * NF],
                                         dbt_sb[kb],
                                         start=(kb == 0), stop=(kb == KB - 1))

            # ---- stage 3: complex pointwise multiply (freq partitions).
            # Vector can read at most one PSUM operand; stage DT in SBUF.
            db_sb = pool.tile([NF, 2, 2 * B], F32, tag="dbsb", name="dbsb")
            for h in range(2):
                nc.vector.tensor_copy(db_sb[:, h, :], db_ps[h][:])
            dr = db_sb[:, 0, :B]
            br = db_sb[:, 0, B:]
            di = db_sb[:, 1, :B]
            bi = db_sb[:, 1, B:]
            t01 = pool.tile([NF, 2, B], F32, tag="t01", name="t01")
            t23 = pool.tile([NF, 2, B], F32, tag="t23", name="t23")
            pt = pool.tile([NF, 2, B], MM_DT, tag="pt", name="pt")
            nc.vector.tensor_mul(t01[:, 0, :], dr, br)
            nc.vector.tensor_mul(t23[:, 0, :], di, bi)
            nc.vector.tensor_mul(t01[:, 1, :], dr, bi)
            nc.vector.tensor_mul(t23[:, 1, :], di, br)
            nc.vector.tensor_sub(pt[:, 0, :], t01[:, 0, :], t23[:, 0, :])
            nc.vector.tensor_add(pt[:, 1, :], t01[:, 1, :], t23[:, 1, :])

            # ---- stage 4: part = PT^T @ G (contraction over s=NF slots) ----
            # Output shipped bf16-packed-in-fp32 (host upcasts and sums).
            out_sb = pool.tile([128, N], BF16 if MM_PREC == "bf16" else F32,
                               tag="outsb", name="outsb")
            last_mm = last_cp = None
            for h in range(2):
                o_ps = pp.tile([128, 512], F32, tag="op", name=f"op{h}", bufs=2)
                for blk in range(2):
                    last_mm = nc.tensor.matmul(
                        o_ps[:],
                        pt[:, blk, :],
                        g_v[:, blk * N + h * 512: blk * N + (h + 1) * 512],
                        start=(blk == 0),
                        stop=(blk == 1),
                    )
                last_cp = nc.vector.tensor_copy(out_sb[:, h * 512:(h + 1) * 512], o_ps[:])
            out_tr = out_sb.bitcast(F32) if MM_PREC == "bf16" else out_sb
            HW = N // PACK // 2
            store_a = nc.sync.dma_start(out[:, :HW], out_tr[:, :HW])
            store_b = nc.scalar.dma_start(out[:, HW:], out_tr[:, HW:])

            # TileContext's exit emits one tail Drain waiting on every
            # outstanding semaphore; walrus caps instructions at ONE sync
            # wait.  Pre-absorb every tick into SP's clock with a chain of
            # single-wait drains so the tail drain needs none.
            prev = None
            for dep in [*in_dmas, warm_dma, store_a, store_b, last_mm, last_cp]:
                dr_i = nc.sync.drain(fusable=False)
                add_dep_helper(dr_i.ins, dep.ins, sync=True,
                               reason="tail: absorb tick into SP clock")
                if prev is not None:
                    add_dep_helper(dr_i.ins, prev.ins, sync=False,
                                   reason="tail: keep drain chain ordered")
                prev = dr_i

    return nc


def _dft_constants():
    """Per-core forward (CC) and inverse (G) DFT matrices, float32.

    CC_c: (N, S) = [cos | -sin] columns for the core's NF slots.
    G_c:  (NF, 2N) row s = [w*cos row | -w*sin row] (cos/sin blocks).
    Slot NF-1 is zero on cores 0..6 and the (real) Nyquist bin on core 7.
    """
    j = np.arange(N, dtype=np.float64)
    ccs, gs = [], []
    for c in range(N_CORES):
        f = np.arange(c * NFREQ, c * NFREQ + NF, dtype=np.float64)
        alive = np.ones(NF)
        wts = np.full(NF, 4.0 / N)
        if c < N_CORES - 1:
            alive[NF - 1] = 0.0
            wts[NF - 1] = 0.0
        else:
            wts[NF - 1] = 2.0 / N     # Nyquist f = N/2
        if c == 0:
            wts[0] = 2.0 / N          # DC
        ang = 2.0 * np.pi * np.outer(j, f) / N            # (N, NF)
        cc = np.concatenate([np.cos(ang) * alive, -np.sin(ang) * alive],
                            axis=1)                        # (N, S)
        angT = ang.T                                       # (NF, N)
        gr = wts[:, None] * np.cos(angT)
        gi = -wts[:, None] * np.sin(angT)
        gmat = np.concatenate([gr, gi], axis=1)            # (NF, 2N)
        ccs.append(np.ascontiguousarray(cc, dtype=np.float32))
        gs.append(np.ascontiguousarray(gmat, dtype=np.float32))
    return ccs, gs


def _partition_pack(a):
    """(R, W) with R = n*128 -> (128, n, W): row p = stack of chunk rows p."""
    r, w = a.shape
    n = r // 128
    return np.ascontiguousarray(a.reshape(n, 128, w).transpose(1, 0, 2))


def kernel(des, body, kernel):
    global LAST_RESULT
    K = np.asarray(kernel, dtype=np.float32)
    kt_np = _partition_pack(_np_in(np.ascontiguousarray(K.T)))   # (128, JC, kw)
    dbt_np = _partition_pack(_np_in(np.concatenate(
        [np.asarray(des, dtype=np.float32).T, np.asarray(body, dtype=np.float32).T],
        axis=1,
    )))  # (128, KB, dw)
    ccs, gs = _dft_constants()

    if "nc" not in _nc_cache:
        _nc_cache["nc"] = _build_nc()
    nc = _nc_cache["nc"]

    JPT = JC // KT_SPLIT
    kw = kt_np.shape[2]
    in_maps = []
    for c in range(N_CORES):
        cc_p = _partition_pack(_np_in(ccs[c]))       # (128, JC, cw)
        ccdbt = np.ascontiguousarray(
            np.concatenate([cc_p, dbt_np], axis=2)).reshape(128, -1)
        g_p = _np_in(gs[c])                          # (NF, gw)
        m = {"ccdbt": ccdbt, "g": np.ascontiguousarray(g_p)}
        for q in range(KT_SPLIT):
            m[f"kt{q}"] = np.ascontiguousarray(
                kt_np[:, q * JPT:(q + 1) * JPT, :].reshape(128, JPT * kw))
        in_maps.append(m)

    res = run_bass_kernel_spmd(nc, in_maps, list(range(N_CORES)))
    LAST_RESULT = res
    out = np.zeros((B, N), dtype=np.float32)
    if MM_PREC == "bf16":
        import ml_dtypes
        for r in res.results:
            raw = np.ascontiguousarray(r["out"]).view(np.uint8)
            out += raw.view(ml_dtypes.bfloat16).reshape(B, N).astype(np.float32)
    else:
        for r in res.results:
            out += r["out"]
    return out


# revision 18
# speedup vs baseline: 1.1461x; 1.1226x over previous
r"""Circulant layer kernel for Trainium2 (8 NeuronCores).

Math: reference computes mv1 + mv2 where
  mv1 = batch_circulant(b) @ d,  mv2 = batch_circulant(d) @ b,
with d = des @ K, b = body @ K.  Both are the circular convolution of d and b
(circular convolution is commutative), so  out = 2 * circconv(d, b).

circconv via DFT:  out = 2 * Re(IDFT(DFT(d) * DFT(b))).
DFT/IDFT are realized as dense matmuls with host-generated constant
cos/sin matrices (input-independent constants).

Real-input symmetry: d, b are real so the spectrum is conjugate-symmetric;
only frequencies 0..512 are computed, with inverse-DFT weights 4/N for
f=1..511 and 2/N for f=0 and f=512 (Nyquist).  513 = 8*64 + 1 frequencies
are spread as 65 slots per core (uniform program): slot 64 is a zero pad on
cores 0..6 and the Nyquist bin on core 7 (whose -sin column is identically
zero, so the complex pointwise multiply degenerates correctly).

Sharding: core c owns frequencies 64c..64c+63 (+pad).  Per core:
  KC_c  = K @ CC_c          (1024k x 130s)   fused projection+forward DFT,
                            j-outer so matmuls ride the K DMA stream
  DT_c  = KC_c^T @ [des^T body^T]  (65s x 256)  x2 (cos/sin blocks)
  PT_c  = complex-mult(DT)  (65s x 128b) x2   on VectorE
  part_c = PT_c^T @ G_c     (128b x 1024)     inverse DFT, contraction s=65
Host sums the 8 partials (unshard).
"""

import numpy as np

import concourse.bass as bass
import concourse.mybir as mybir
import concourse.tile as tile
from concourse.bass_utils import run_bass_kernel_spmd
from concourse.tile_rust import add_dep_helper

B = 128        # batch
D_IN = 1024    # input feature dim (contraction k)
N = 1024       # output feature dim (conv length j) == #frequencies
N_CORES = 8
NFREQ = 64          # real frequencies per core
NF = NFREQ + 1      # slots per block (incl pad/Nyquist)
S = 2 * NF          # total freq slots per core: [0:NF]=cos, [NF:2NF]=-sin

F32 = mybir.dt.float32
F32R = mybir.dt.float32r
BF16 = mybir.dt.bfloat16

# Matmul operand precision: "bf16" (fastest; ~5e-3 rel err), "f32r"
# (single-pass TF32-like; ~3e-4), "f32" (two-pass full fp32; ~7e-7).
import os as _os
MM_PREC = _os.environ.get("CIRC_MM_PREC", "bf16")
MM_DT = {"bf16": BF16, "f32r": F32R, "f32": F32}[MM_PREC]


def _np_in(a):
    """Cast to the matmul precision; bf16 data is shipped packed in fp32
    words (DMA is element-rate-bound: 2-byte elements run at half rate)."""
    import ml_dtypes
    a = np.ascontiguousarray(np.asarray(a, dtype=np.float32))
    if MM_PREC != "bf16":
        return a
    bf = np.ascontiguousarray(a.astype(ml_dtypes.bfloat16))
    return bf.view(np.uint8).reshape(a.shape[0], -1).view(np.float32)

# Number of fp32 transport words per logical input element.
PACK = 2 if MM_PREC == "bf16" else 1
# Transport dtype: bf16 ships packed in fp32 words; f32/f32r ship natively
# (the fp32r verifier requires the producing DMA to be f32r-typed).
TR_DT = F32 if MM_PREC == "bf16" else MM_DT

# Stashed by kernel() for test harnesses that want profiling info.
LAST_RESULT = None

_nc_cache = {}

JC = N // 128      # 8 chunks over j (contraction of KC stage)
KB = D_IN // 128   # 8 blocks over k (output partitions of KC stage)
KT_SPLIT = 4       # K^T arrives in this many DMA transfers (j-chunk pairs)


def _build_nc():
    """Build the (single-program) Bass module run on all 8 cores."""
    nc = bass.Bass(target_bir_lowering=True)

    # Input packing (all host-packed per SBUF partition; every DMA moves
    # long contiguous rows — DMA throughput is descriptor-rate-bound
    # otherwise).  All input transfers ride ONE serial SP chain: parallel
    # channels all pay the full proxy latency, while a serial chain
    # pipelines.  Transfer order = consumption order:
    #   ccdbt[p, ic, :]  = CC chunk ic rows | des^T/body^T chunk ic rows
    #   kt{q}[p, h, :]   = K^T rows for j-chunk 2q+h  (stage-1 stream)
    #   g[s, blk, :]     = inverse-DFT rows (cos/sin blocks), s=0..NF-1
    CW = S // PACK            # CC words per chunk row
    DW = 2 * B // PACK        # dbt words per chunk row
    GW = 2 * N // PACK        # g words per slot row (cos+sin blocks)
    cc_par = nc.declare_dram_parameter("cc", [128, JC * CW], TR_DT, False)
    kt_q = [nc.declare_dram_parameter(f"kt{q}", [128, (JC // KT_SPLIT) * D_IN // PACK],
                                      TR_DT, False)
            for q in range(KT_SPLIT)]
    # aux row p = dbt row p | g row p (g rows 65..127 are padding: only 8 HW
    # DMA queues exist and each DMACopy may carry one sync wait, so inputs
    # are capped at 6 transfers + 2 output stores).
    aux_par = nc.declare_dram_parameter("aux", [128, KB * DW + GW], TR_DT, False)
    out = nc.declare_dram_parameter("out", [B, N // PACK], F32, isOutput=True)

    JPT = JC // KT_SPLIT   # j-chunks per kt transfer

    with tile.TileContext(nc) as tc:
        with (
            tc.tile_pool(name="main", bufs=1) as pool,
            tc.tile_pool(name="psum", bufs=1, space="PSUM") as pp,
        ):
            # ---- inputs -> SBUF: two parallel HWDGE chains (SP + ACT),
            # each serial-FIFO, ordered by consumption: CC first (tiny) so
            # stage 1 starts at the first K^T chunk, K^T split across both
            # chains, dbt+g (needed last) at the back of the SP chain.
            in_dmas = []
            cc_st = pool.tile([128, JC, CW], TR_DT, tag="cc", name="cc")
            in_dmas.append(nc.sync.dma_start(cc_st[:], cc_par[:, :]))
            kt_sb = [pool.tile([128, JPT, D_IN // PACK], TR_DT, tag=f"kt{q}", name=f"kt{q}")
                     for q in range(KT_SPLIT)]
            for q in range(KT_SPLIT):
                eng = nc.sync if q % 2 == 0 else nc.scalar
                in_dmas.append(eng.dma_start(kt_sb[q][:], kt_q[q][:, :]))
            aux_sb = pool.tile([128, KB * DW + GW], TR_DT, tag="aux", name="aux")
            in_dmas.append(nc.sync.dma_start(aux_sb[:], aux_par[:, :]))

            cc_v = cc_st.bitcast(MM_DT)                # [128, JC, S]
            cc_sb = [cc_v[:, j, :] for j in range(JC)]
            aux_v = aux_sb.bitcast(MM_DT)              # [128, KB*2B + 2N]
            dbt_sb = [aux_v[:, kb * 2 * B:(kb + 1) * 2 * B] for kb in range(KB)]
            kt_v = [t.bitcast(MM_DT) for t in kt_sb]   # [128, JPT, D_IN]
            g_v = aux_v[:NF, KB * 2 * B:]              # [NF, 2N]

            # ---- PE warmup: keep the HAM clock un-throttled while DMAs
            # stream in, so the real matmuls all run at 2.4 GHz. Dead-code
            # proofed by a tiny gpsimd DMA of the result to scratch DRAM.
            # Dead-code proofed below by adding wps (exactly 0.0) into a
            # live tensor — no scratch DMA, keeping total DMACopies at 8.
            wz = pool.tile([128, 640], BF16, tag="wz", name="wz")
            nc.gpsimd.memset(wz[:], 0.0)
            wps = pp.tile([128, 512], F32, tag="op", name="wps", bufs=2)
            for w in range(16):
                nc.tensor.matmul(wps[:], wz[:, :128], wz[:, 128:640],
                                 start=True, stop=True)

            # ---- stage 1, j-outer: KC[k, s] = sum_j KT[j, k] * CC[j, s].
            # All 8 k-blocks accumulate in PSUM simultaneously so each
            # j-chunk's matmuls issue as soon as that chunk's K^T lands.
            # PSUM is 8 banks of 2KB/partition and accumulation groups may
            # not share a bank, so stage 1 runs in two passes of 4 k-blocks
            # (4 banks, reused), each pass j-outer over the K^T stream.
            # Stage 2 consumes each k-block right after its group closes.
            kc_ps = [pp.tile([128, S], F32, tag=f"kcp{q}", name=f"kcp{q}")
                     for q in range(4)]
            kc_sb = [pool.tile([128, S], MM_DT, tag=f"kc{kb}", name=f"kc{kb}")
                     for kb in range(KB)]
            db_ps = [pp.tile([NF, 2 * B], F32, tag=f"dbp{h}", name=f"dbp{h}")
                     for h in range(2)]
            # j-chunks processed in DMA-arrival order (scalar chain's first
            # transfer tends to land before the SP chain's second).
            jorder = [2, 3, 0, 1, 6, 7, 4, 5]
            for half in range(2):
                for idx, j in enumerate(jorder):
                    ktj = kt_v[j // JPT][:, j % JPT, :]
                    for q in range(4):
                        kb = half * 4 + q
                        nc.tensor.matmul(
                            kc_ps[q][:],
                            ktj[:, kb * 128:(kb + 1) * 128],
                            cc_sb[j],
                            start=(idx == 0),
                            stop=(idx == JC - 1),
                        )
                for q in range(4):
                    kb = half * 4 + q
                    nc.vector.tensor_copy(kc_sb[kb][:], kc_ps[q][:])
            for kb in range(KB):
                for h in range(2):
                    nc.tensor.matmul(db_ps[h][:],
                                     kc_sb[kb][:, h * NF:(h + 1) * NF],
                                     dbt_sb[kb],
                                     start=(kb == 0), stop=(kb == KB - 1))

            # ---- stage 3: complex pointwise multiply (freq partitions).
            # Vector can read at most one PSUM operand; stage DT in SBUF.
            db_sb = pool.tile([NF, 2, 2 * B], F32, tag="dbsb", name="dbsb")
            for h in range(2):
                nc.vector.tensor_copy(db_sb[:, h, :], db_ps[h][:])
            # consume the warmup result (exactly zero) so it isn't DCE'd
            nc.vector.tensor_add(db_sb[:, 0, :4], db_sb[:, 0, :4], wps[:NF, :4])
            dr = db_sb[:, 0, :B]
            br = db_sb[:, 0, B:]
            di = db_sb[:, 1, :B]
            bi = db_sb[:, 1, B:]
            t01 = pool.tile([NF, 2, B], F32, tag="t01", name="t01")
            t23 = pool.tile([NF, 2, B], F32, tag="t23", name="t23")
            pt = pool.tile([NF, 2, B], MM_DT, tag="pt", name="pt")
            nc.vector.tensor_mul(t01[:, 0, :], dr, br)
            nc.vector.tensor_mul(t23[:, 0, :], di, bi)
            nc.vector.tensor_mul(t01[:, 1, :], dr, bi)
            nc.vector.tensor_mul(t23[:, 1, :], di, br)
            nc.vector.tensor_sub(pt[:, 0, :], t01[:, 0, :], t23[:, 0, :])
            nc.vector.tensor_add(pt[:, 1, :], t01[:, 1, :], t23[:, 1, :])

            # ---- stage 4: part = PT^T @ G (contraction over s=NF slots) ----
            # Output shipped bf16-packed-in-fp32 (host upcasts and sums).
            out_sb = pool.tile([128, N], BF16 if MM_PREC == "bf16" else F32,
                               tag="outsb", name="outsb")
            out_tr = out_sb.bitcast(F32) if MM_PREC == "bf16" else out_sb
            HW = N // PACK // 2
            stores = []
            last_mm = last_cp = None
            for h in range(2):
                o_ps = pp.tile([128, 512], F32, tag="op", name=f"op{h}", bufs=2)
                for blk in range(2):
                    last_mm = nc.tensor.matmul(
                        o_ps[:],
                        pt[:, blk, :],
                        g_v[:, blk * N + h * 512: blk * N + (h + 1) * 512],
                        start=(blk == 0),
                        stop=(blk == 1),
                    )
                last_cp = nc.vector.tensor_copy(out_sb[:, h * 512:(h + 1) * 512], o_ps[:])
                eng = nc.sync if h == 0 else nc.scalar
                stores.append(eng.dma_start(out[:, h * HW:(h + 1) * HW],
                                            out_tr[:, h * HW:(h + 1) * HW]))
            store_a, store_b = stores

            # TileContext's exit emits one tail Drain waiting on every
            # outstanding semaphore; walrus caps instructions at ONE sync
            # wait.  Pre-absorb every tick into SP's clock with a chain of
            # single-wait drains so the tail drain needs none.
            prev = None
            for dep in [*in_dmas, store_a, store_b, last_mm, last_cp]:
                dr_i = nc.sync.drain(fusable=False)
                add_dep_helper(dr_i.ins, dep.ins, sync=True,
                               reason="tail: absorb tick into SP clock")
                if prev is not None:
                    add_dep_helper(dr_i.ins, prev.ins, sync=False,
                                   reason="tail: keep drain chain ordered")
                prev = dr_i

    return nc


def _dft_constants():
    """Per-core forward (CC) and inverse (G) DFT matrices, float32.

    CC_c: (N, S) = [cos | -sin] columns for the core's NF slots.
    G_c:  (NF, 2N) row s = [w*cos row | -w*sin row] (cos/sin blocks).
    Slot NF-1 is zero on cores 0..6 and the (real) Nyquist bin on core 7.
    """
    j = np.arange(N, dtype=np.float64)
    ccs, gs = [], []
    for c in range(N_CORES):
        f = np.arange(c * NFREQ, c * NFREQ + NF, dtype=np.float64)
        alive = np.ones(NF)
        wts = np.full(NF, 4.0 / N)
        if c < N_CORES - 1:
            alive[NF - 1] = 0.0
            wts[NF - 1] = 0.0
        else:
            wts[NF - 1] = 2.0 / N     # Nyquist f = N/2
        if c == 0:
            wts[0] = 2.0 / N          # DC
        ang = 2.0 * np.pi * np.outer(j, f) / N            # (N, NF)
        cc = np.concatenate([np.cos(ang) * alive, -np.sin(ang) * alive],
                            axis=1)                        # (N, S)
        angT = ang.T                                       # (NF, N)
        gr = wts[:, None] * np.cos(angT)
        gi = -wts[:, None] * np.sin(angT)
        gmat = np.concatenate([gr, gi], axis=1)            # (NF, 2N)
        ccs.append(np.ascontiguousarray(cc, dtype=np.float32))
        gs.append(np.ascontiguousarray(gmat, dtype=np.float32))
    return ccs, gs


def _partition_pack(a):
    """(R, W) with R = n*128 -> (128, n, W): row p = stack of chunk rows p."""
    r, w = a.shape
    n = r // 128
    return np.ascontiguousarray(a.reshape(n, 128, w).transpose(1, 0, 2))


def kernel(des, body, kernel):
    global LAST_RESULT
    K = np.asarray(kernel, dtype=np.float32)
    kt_np = _partition_pack(_np_in(np.ascontiguousarray(K.T)))   # (128, JC, kw)
    dbt_np = _partition_pack(_np_in(np.concatenate(
        [np.asarray(des, dtype=np.float32).T, np.asarray(body, dtype=np.float32).T],
        axis=1,
    )))  # (128, KB, dw)
    ccs, gs = _dft_constants()

    if "nc" not in _nc_cache:
        _nc_cache["nc"] = _build_nc()
    nc = _nc_cache["nc"]

    JPT = JC // KT_SPLIT
    kw = kt_np.shape[2]
    dbt_flat = dbt_np.reshape(128, -1)
    kt_flat = [np.ascontiguousarray(
        kt_np[:, q * JPT:(q + 1) * JPT, :].reshape(128, JPT * kw))
        for q in range(KT_SPLIT)]
    in_maps = []
    for c in range(N_CORES):
        cc_p = np.ascontiguousarray(
            _partition_pack(_np_in(ccs[c])).reshape(128, -1))
        g_p = _np_in(gs[c])                          # (NF, gw)
        g_pad = np.zeros((128, g_p.shape[1]), dtype=np.float32)
        g_pad[:NF] = g_p
        aux = np.ascontiguousarray(
            np.concatenate([dbt_flat, g_pad], axis=1))
        m = {"cc": cc_p, "aux": aux}
        for q in range(KT_SPLIT):
            m[f"kt{q}"] = kt_flat[q]
        in_maps.append(m)

    res = run_bass_kernel_spmd(nc, in_maps, list(range(N_CORES)))
    LAST_RESULT = res
    out = np.zeros((B, N), dtype=np.float32)
    if MM_PREC == "bf16":
        import ml_dtypes
        for r in res.results:
            raw = np.ascontiguousarray(r["out"]).view(np.uint8)
            out += raw.view(ml_dtypes.bfloat16).reshape(B, N).astype(np.float32)
    else:
        for r in res.results:
            out += r["out"]
    return out
